# revision 2
# baseline (speedup 1.0000x reference)
"""Trainium2 Bass kernel for ContextQueryAttention (trilinear attention), v2.

Math (per batch b; C:[D,N], Q:[D,M], W0:[3D]=[w_q|w_c|w_qc], b0):
    S[n,m] = cs[n] + qs[m] + sum_d C[d,n]*w_qc[d]*Q[d,m] + b0
      with cs = Ct@w_c, qs = Qt@w_q
    S_row = softmax_m(S), S_col = softmax_n(S)
    A  = S_row @ Qt                  # (N, D)
    Bt = S_row @ (S_col^T @ Ct)      # (N, D), N x N intermediate dropped

Restructurings vs the fp32 baseline:
  * Bias folding: the X matmul rhs is Q*w_qc + w_c (per-partition fused
    multiply-add), so its output is X[n,m] + cs[n] directly; likewise the
    X^T rhs is C*w_qc + w_q giving X^T[m,n] + qs[m]. exp() then needs no
    per-chunk bias -> 4 big ACT instructions per batch instead of 80 small
    ones, and no bias copies.
  * softmax_m is invariant to per-row constants, softmax_n to per-column
    constants, so e_col = exp(X+cs) serves the col path and
    e_row = exp(X^T+qs) the row path; b0 cancels everywhere.
  * Input magnitudes are O(5): exp() needs no max-subtraction.
  * Softmax denominators ride along as all-ones columns fused into the
    consuming matmuls; normalization is a per-partition scalar multiply on
    the PSUM->SBUF copy.
  * All matmuls in bf16 (full-rate, FWL weight loads, odd moving sizes ok);
    fp32 PSUM accumulation keeps the error ~1e-3 << 2e-2 gate.
  * Outputs stored/DMA'd as bf16 (one DMA per tensor per batch), widened to
    fp32 on host.

Sharding: data-parallel over batch, 8 batches per core on 8 cores.
"""

import numpy as np

import concourse.bass as bass
import concourse.bacc as bacc
import concourse.tile as tile
from concourse import mybir
from concourse.bass_utils import run_bass_kernel_spmd
from concourse.masks import make_identity

F32 = mybir.dt.float32
BF16 = mybir.dt.bfloat16
MUL = mybir.AluOpType.mult
ADD = mybir.AluOpType.add
EXP = mybir.ActivationFunctionType.Exp

# Problem shape (hardcoded per spec)
B, D, N, M = 64, 128, 1024, 256
NCORES = 8
BPC = B // NCORES  # batches per core
NK = N // 128      # context chunks (8)
MJ = M // 128      # query chunks (2)


def build_kernel(bpc: int = BPC) -> bass.Bass:
    nc = bacc.Bacc("TRN2", target_bir_lowering=False, debug=False)

    C8 = nc.dram_tensor("C", [bpc, D, N], F32, kind="ExternalInput").ap()
    Q8 = nc.dram_tensor("Q", [bpc, D, M], F32, kind="ExternalInput").ap()
    W0 = nc.dram_tensor("W0", [3 * D], F32, kind="ExternalInput").ap()
    A8 = nc.dram_tensor("A", [bpc, N, D], BF16, kind="ExternalOutput").ap()
    B8 = nc.dram_tensor("Bt", [bpc, N, D], BF16, kind="ExternalOutput").ap()

    with tile.TileContext(nc) as tc:
        with (
            tc.tile_pool(name="singles", bufs=1) as singles,
            tc.tile_pool(name="inp", bufs=2) as pool_in,
            tc.tile_pool(name="b16", bufs=2) as pool_b16,
            tc.tile_pool(name="e", bufs=2) as pool_e,
            tc.tile_pool(name="tq", bufs=2) as pool_tq,
            tc.tile_pool(name="sm", bufs=2) as pool_sm,
            tc.tile_pool(name="out", bufs=2) as pool_out,
            # PSUM: 8 banks total.
            # pp_big: 2-bank slots x2 for the X / X^T score matmuls.
            # pp_t: transpose staging; pp_gr: col(G) + row(A|Bt) accumulators.
            tc.tile_pool(name="pp_big", bufs=2, space="PSUM") as pp_big,
            tc.tile_pool(name="pp_t", bufs=2, space="PSUM") as pp_t,
            tc.tile_pool(name="pp_gr", bufs=2, space="PSUM") as pp_gr,
        ):
            # --- constants ---
            # wvec [128, 3] = [w_q | w_c | w_qc], one DMA
            wvec = singles.tile([D, 3], F32)
            nc.sync.dma_start(out=wvec, in_=W0.rearrange("(a p) -> p a", p=D))
            w_q = wvec[:, 0:1]
            w_c = wvec[:, 1:2]
            w_qc = wvec[:, 2:3]
            ones_ct = singles.tile([128, NK, 2], BF16)
            nc.vector.memset(ones_ct, 1.0)
            ones_qt = singles.tile([128, MJ, 2], BF16)
            nc.vector.memset(ones_qt, 1.0)
            ident = singles.tile([128, 128], BF16)
            make_identity(nc, ident)

            for b in range(bpc):
                cb = pool_in.tile([D, N], F32, tag="cb")
                qb = pool_in.tile([D, M], F32, tag="qb")
                for h in range(2):
                    nc.sync.dma_start(
                        out=cb[:, h * (N // 2) : (h + 1) * (N // 2)],
                        in_=C8[b, :, h * (N // 2) : (h + 1) * (N // 2)],
                    )
                nc.sync.dma_start(out=qb, in_=Q8[b])

                # bf16 copies (GpSimd - otherwise idle) and fused-bias scaled
                # versions (DVE):
                #   cswq = C*w_qc + w_q  -> X^T matmul rhs (adds qs[m])
                #   qswc = Q*w_qc + w_c  -> X matmul rhs  (adds cs[n])
                cb16 = pool_b16.tile([D, N], BF16, tag="cb16")
                qb16 = pool_b16.tile([D, M], BF16, tag="qb16")
                for h in range(2):
                    nc.gpsimd.tensor_copy(
                        out=cb16[:, h * (N // 2) : (h + 1) * (N // 2)],
                        in_=cb[:, h * (N // 2) : (h + 1) * (N // 2)],
                    )
                nc.gpsimd.tensor_copy(out=qb16, in_=qb)
                cswq = pool_b16.tile([D, N], BF16, tag="cswq")
                qswc = pool_b16.tile([D, M], BF16, tag="qswc")
                for h in range(2):
                    nc.gpsimd.tensor_scalar(
                        out=cswq[:, h * (N // 2) : (h + 1) * (N // 2)],
                        in0=cb[:, h * (N // 2) : (h + 1) * (N // 2)],
                        scalar1=w_qc,
                        scalar2=w_q,
                        op0=MUL,
                        op1=ADD,
                    )
                nc.gpsimd.tensor_scalar(
                    out=qswc, in0=qb, scalar1=w_qc, scalar2=w_c, op0=MUL, op1=ADD
                )

                # --- X path: px[n-chunk, m] = X + cs, then e_col = exp ---
                e_col = pool_e.tile([128, NK, M], BF16, tag="e_col")
                for h in range(2):  # halves of the n-chunks
                    px = pp_big.tile([128, NK // 2, M], F32, tag="pbig")
                    for kk in range(NK // 2):
                        k = h * (NK // 2) + kk
                        nc.tensor.matmul(
                            px[:, kk, :],
                            cb16[:, k * 128 : (k + 1) * 128],
                            qswc,
                            start=True,
                            stop=True,
                        )
                    nc.scalar.activation(
                        out=e_col[:, h * (NK // 2) : (h + 1) * (NK // 2), :],
                        in_=px,
                        func=EXP,
                    )

                # --- transposes: ct_k = [Ct_k | 1 1], qtg_j = [Qt_j |1 1| G_j]
                ct = pool_tq.tile([128, NK, D + 2], BF16, tag="ct")
                nc.vector.tensor_copy(out=ct[:, :, D : D + 2], in_=ones_ct)
                for g in range(NK // 4):
                    pt = pp_t.tile([128, 4, 128], BF16, tag="pt")
                    for kk in range(4):
                        k = g * 4 + kk
                        nc.tensor.transpose(
                            pt[:, kk, :], cb16[:, k * 128 : (k + 1) * 128], ident
                        )
                    nc.vector.tensor_copy(out=ct[:, g * 4 : (g + 1) * 4, 0:D], in_=pt)

                qtg = pool_tq.tile([128, MJ, 2 * D + 2], BF16, tag="qtg")
                nc.vector.tensor_copy(out=qtg[:, :, D : D + 2], in_=ones_qt)
                pt = pp_t.tile([128, 2, 128], BF16, tag="pt")
                for j in range(MJ):
                    nc.tensor.transpose(
                        pt[:, j, :], qb16[:, j * 128 : (j + 1) * 128], ident
                    )
                nc.vector.tensor_copy(out=qtg[:, :, 0:D], in_=pt)

                # --- X^T path: pxt[m-chunk, n] = X^T + qs, e_row = exp ---
                e_row = pool_e.tile([128, MJ, N], BF16, tag="e_row")
                for j in range(MJ):
                    pxt = pp_big.tile([128, N], F32, tag="pbig")
                    for h in range(N // 512):
                        nc.tensor.matmul(
                            pxt[:, h * 512 : (h + 1) * 512],
                            qb16[:, j * 128 : (j + 1) * 128],
                            cswq[:, h * 512 : (h + 1) * 512],
                            start=True,
                            stop=True,
                        )
                    nc.scalar.activation(out=e_row[:, j, :], in_=pxt, func=EXP)

                # --- col path: G_j = normalize(e_col^T @ [Ct|1 1]) ---
                for j in range(MJ):
                    pg = pp_gr.tile([128, D + 2], F32, tag="pgr")
                    for k in range(NK):
                        nc.tensor.matmul(
                            pg,
                            e_col[:, k, j * 128 : (j + 1) * 128],
                            ct[:, k, :],
                            start=(k == 0),
                            stop=(k == NK - 1),
                        )
                    rcol = pool_sm.tile([128, 1], F32, tag=f"rcol{j}")
                    nc.vector.reciprocal(out=rcol, in_=pg[:, D : D + 1])
                    nc.vector.tensor_scalar_mul(
                        out=qtg[:, j, D + 2 : 2 * D + 2], in0=pg[:, 0:D], scalar1=rcol
                    )

                # --- row path: [A |rowsum rowsum| Bt] = e_row^T @ [Qt|1 1|G]
                oab = pool_out.tile([128, NK, 2 * D + 2], BF16, tag="oab")
                for k in range(NK):
                    pab = pp_gr.tile([128, 2 * D + 2], F32, tag="pgr")
                    for j in range(MJ):
                        nc.tensor.matmul(
                            pab,
                            e_row[:, j, k * 128 : (k + 1) * 128],
                            qtg[:, j, :],
                            start=(j == 0),
                            stop=(j == MJ - 1),
                        )
                    rrow = pool_sm.tile([128, 1], F32, tag=f"rrow{k}")
                    nc.vector.reciprocal(out=rrow, in_=pab[:, D : D + 1])
                    nc.vector.tensor_scalar_mul(
                        out=oab[:, k, :], in0=pab, scalar1=rrow
                    )
                    if k % 4 == 3:
                        g0, g1 = k - 3, k + 1
                        nc.sync.dma_start(
                            out=A8[b].rearrange("(k p) d -> p k d", p=128)[
                                :, g0:g1, :
                            ],
                            in_=oab[:, g0:g1, 0:D],
                        )
                        nc.sync.dma_start(
                            out=B8[b].rearrange("(k p) d -> p k d", p=128)[
                                :, g0:g1, :
                            ],
                            in_=oab[:, g0:g1, D + 2 : 2 * D + 2],
                        )
    nc.finalize()
    return nc


_NC_CACHE = None


def kernel(C, Q, W0, b0, _trace=False):
    global _NC_CACHE
    if _NC_CACHE is None:
        _NC_CACHE = build_kernel()
    nc = _NC_CACHE

    C = np.ascontiguousarray(np.asarray(C, dtype=np.float32))
    Q = np.ascontiguousarray(np.asarray(Q, dtype=np.float32))
    W0 = np.ascontiguousarray(np.asarray(W0, dtype=np.float32))

    in_maps = [
        {
            "C": C[i * BPC : (i + 1) * BPC],
            "Q": Q[i * BPC : (i + 1) * BPC],
            "W0": W0,
        }
        for i in range(NCORES)
    ]
    res = run_bass_kernel_spmd(nc, in_maps, core_ids=list(range(NCORES)))
    A = np.concatenate(
        [res.results[i]["A"] for i in range(NCORES)], axis=0
    ).astype(np.float32)
    Bt = np.concatenate(
        [res.results[i]["Bt"] for i in range(NCORES)], axis=0
    ).astype(np.float32)
    return (A, Bt)


# revision 3
# speedup vs baseline: 1.1262x; 1.1262x over previous
"""Trainium2 Bass kernel for ContextQueryAttention (trilinear attention), v2.

Math (per batch b; C:[D,N], Q:[D,M], W0:[3D]=[w_q|w_c|w_qc], b0):
    S[n,m] = cs[n] + qs[m] + sum_d C[d,n]*w_qc[d]*Q[d,m] + b0
      with cs = Ct@w_c, qs = Qt@w_q
    S_row = softmax_m(S), S_col = softmax_n(S)
    A  = S_row @ Qt                  # (N, D)
    Bt = S_row @ (S_col^T @ Ct)      # (N, D), N x N intermediate dropped

Restructurings vs the fp32 baseline:
  * Bias folding: the X matmul rhs is Q*w_qc + w_c (per-partition fused
    multiply-add), so its output is X[n,m] + cs[n] directly; likewise the
    X^T rhs is C*w_qc + w_q giving X^T[m,n] + qs[m]. exp() then needs no
    per-chunk bias -> 4 big ACT instructions per batch instead of 80 small
    ones, and no bias copies.
  * softmax_m is invariant to per-row constants, softmax_n to per-column
    constants, so e_col = exp(X+cs) serves the col path and
    e_row = exp(X^T+qs) the row path; b0 cancels everywhere.
  * Input magnitudes are O(5): exp() needs no max-subtraction.
  * Softmax denominators ride along as all-ones columns fused into the
    consuming matmuls; normalization is a per-partition scalar multiply on
    the PSUM->SBUF copy.
  * All matmuls in bf16 (full-rate, FWL weight loads, odd moving sizes ok);
    fp32 PSUM accumulation keeps the error ~1e-3 << 2e-2 gate.
  * Outputs stored/DMA'd as bf16 (one DMA per tensor per batch), widened to
    fp32 on host.

Sharding: data-parallel over batch, 8 batches per core on 8 cores.
"""

import numpy as np

import concourse.bass as bass
import concourse.bacc as bacc
import concourse.tile as tile
from concourse import mybir
from concourse.bass_utils import run_bass_kernel_spmd
from concourse.masks import make_identity

F32 = mybir.dt.float32
BF16 = mybir.dt.bfloat16
MUL = mybir.AluOpType.mult
ADD = mybir.AluOpType.add
EXP = mybir.ActivationFunctionType.Exp

# Problem shape (hardcoded per spec)
B, D, N, M = 64, 128, 1024, 256
NCORES = 8
BPC = B // NCORES  # batches per core
NK = N // 128      # context chunks (8)
MJ = M // 128      # query chunks (2)


def build_kernel(bpc: int = BPC) -> bass.Bass:
    nc = bacc.Bacc("TRN2", target_bir_lowering=False, debug=False)

    C8 = nc.dram_tensor("C", [bpc, D, N], F32, kind="ExternalInput").ap()
    Q8 = nc.dram_tensor("Q", [bpc, D, M], F32, kind="ExternalInput").ap()
    W0 = nc.dram_tensor("W0", [3 * D], F32, kind="ExternalInput").ap()
    A8 = nc.dram_tensor("A", [bpc, N, D], BF16, kind="ExternalOutput").ap()
    B8 = nc.dram_tensor("Bt", [bpc, N, D], BF16, kind="ExternalOutput").ap()

    with tile.TileContext(nc) as tc:
        with (
            tc.tile_pool(name="singles", bufs=1) as singles,
            tc.tile_pool(name="inp", bufs=2) as pool_in,
            tc.tile_pool(name="b16", bufs=2) as pool_b16,
            tc.tile_pool(name="e", bufs=2) as pool_e,
            tc.tile_pool(name="tq", bufs=2) as pool_tq,
            tc.tile_pool(name="sm", bufs=2) as pool_sm,
            tc.tile_pool(name="out", bufs=2) as pool_out,
            # PSUM: 8 banks total.
            # pp_big: 2-bank slots x2 for the X / X^T score matmuls.
            # pp_t: transpose staging; pp_gr: col(G) + row(A|Bt) accumulators.
            tc.tile_pool(name="pp_big", bufs=2, space="PSUM") as pp_big,
            tc.tile_pool(name="pp_t", bufs=2, space="PSUM") as pp_t,
            tc.tile_pool(name="pp_gr", bufs=2, space="PSUM") as pp_gr,
        ):
            # --- constants ---
            # wvec [128, 3] = [w_q | w_c | w_qc], one DMA
            wvec = singles.tile([D, 3], F32)
            nc.sync.dma_start(out=wvec, in_=W0.rearrange("(a p) -> p a", p=D))
            w_q = wvec[:, 0:1]
            w_c = wvec[:, 1:2]
            w_qc = wvec[:, 2:3]
            ones_ct = singles.tile([128, NK, 2], BF16)
            nc.vector.memset(ones_ct, 1.0)
            ones_qt = singles.tile([128, MJ, 2], BF16)
            nc.vector.memset(ones_qt, 1.0)
            ident = singles.tile([128, 128], BF16)
            make_identity(nc, ident)

            for b in range(bpc):
                cb = pool_in.tile([D, N], F32, tag="cb")
                qb = pool_in.tile([D, M], F32, tag="qb")
                nc.sync.dma_start(out=qb, in_=Q8[b])
                for h in range(2):
                    nc.sync.dma_start(
                        out=cb[:, h * (N // 2) : (h + 1) * (N // 2)],
                        in_=C8[b, :, h * (N // 2) : (h + 1) * (N // 2)],
                    )

                # bf16 copies (GpSimd - otherwise idle) and fused-bias scaled
                # versions (DVE):
                #   cswq = C*w_qc + w_q  -> X^T matmul rhs (adds qs[m])
                #   qswc = Q*w_qc + w_c  -> X matmul rhs  (adds cs[n])
                cb16 = pool_b16.tile([D, N], BF16, tag="cb16")
                qb16 = pool_b16.tile([D, M], BF16, tag="qb16")
                cswq = pool_b16.tile([D, N], BF16, tag="cswq")
                qswc = pool_b16.tile([D, M], BF16, tag="qswc")
                nc.gpsimd.tensor_scalar(
                    out=qswc, in0=qb, scalar1=w_qc, scalar2=w_c, op0=MUL, op1=ADD
                )
                nc.gpsimd.tensor_copy(out=qb16, in_=qb)
                for h in range(2):
                    nc.gpsimd.tensor_copy(
                        out=cb16[:, h * (N // 2) : (h + 1) * (N // 2)],
                        in_=cb[:, h * (N // 2) : (h + 1) * (N // 2)],
                    )
                for h in range(2):
                    nc.gpsimd.tensor_scalar(
                        out=cswq[:, h * (N // 2) : (h + 1) * (N // 2)],
                        in0=cb[:, h * (N // 2) : (h + 1) * (N // 2)],
                        scalar1=w_qc,
                        scalar2=w_q,
                        op0=MUL,
                        op1=ADD,
                    )

                # --- X path: px[n-chunk, m] = X + cs, then e_col = exp ---
                e_col = pool_e.tile([128, NK, M], BF16, tag="e_col")
                for h in range(2):  # halves of the n-chunks
                    px = pp_big.tile([128, NK // 2, M], F32, tag="pbig")
                    for kk in range(NK // 2):
                        k = h * (NK // 2) + kk
                        nc.tensor.matmul(
                            px[:, kk, :],
                            cb16[:, k * 128 : (k + 1) * 128],
                            qswc,
                            start=True,
                            stop=True,
                        )
                    nc.scalar.activation(
                        out=e_col[:, h * (NK // 2) : (h + 1) * (NK // 2), :],
                        in_=px,
                        func=EXP,
                    )

                # --- transposes: ct_k = [Ct_k | 1 1], qtg_j = [Qt_j |1 1| G_j]
                ct = pool_tq.tile([128, NK, D + 2], BF16, tag="ct")
                nc.vector.tensor_copy(out=ct[:, :, D : D + 2], in_=ones_ct)
                for g in range(NK // 4):
                    pt = pp_t.tile([128, 4, 128], BF16, tag="pt")
                    for kk in range(4):
                        k = g * 4 + kk
                        nc.tensor.transpose(
                            pt[:, kk, :], cb16[:, k * 128 : (k + 1) * 128], ident
                        )
                    nc.vector.tensor_copy(out=ct[:, g * 4 : (g + 1) * 4, 0:D], in_=pt)

                qtg = pool_tq.tile([128, MJ, 2 * D + 2], BF16, tag="qtg")
                nc.vector.tensor_copy(out=qtg[:, :, D : D + 2], in_=ones_qt)
                pt = pp_t.tile([128, 2, 128], BF16, tag="pt")
                for j in range(MJ):
                    nc.tensor.transpose(
                        pt[:, j, :], qb16[:, j * 128 : (j + 1) * 128], ident
                    )
                nc.vector.tensor_copy(out=qtg[:, :, 0:D], in_=pt)

                # --- X^T path: pxt[m-chunk, n] = X^T + qs, e_row = exp ---
                e_row = pool_e.tile([128, MJ, N], BF16, tag="e_row")
                for j in range(MJ):
                    pxt = pp_big.tile([128, N], F32, tag="pbig")
                    for h in range(N // 512):
                        nc.tensor.matmul(
                            pxt[:, h * 512 : (h + 1) * 512],
                            qb16[:, j * 128 : (j + 1) * 128],
                            cswq[:, h * 512 : (h + 1) * 512],
                            start=True,
                            stop=True,
                        )
                    nc.scalar.activation(out=e_row[:, j, :], in_=pxt, func=EXP)

                # --- col path: G_j = normalize(e_col^T @ [Ct|1 1]) ---
                for j in range(MJ):
                    pg = pp_gr.tile([128, D + 2], F32, tag="pgr")
                    for k in range(NK):
                        nc.tensor.matmul(
                            pg,
                            e_col[:, k, j * 128 : (j + 1) * 128],
                            ct[:, k, :],
                            start=(k == 0),
                            stop=(k == NK - 1),
                        )
                    rcol = pool_sm.tile([128, 1], F32, tag=f"rcol{j}")
                    nc.vector.reciprocal(out=rcol, in_=pg[:, D : D + 1])
                    nc.vector.tensor_scalar_mul(
                        out=qtg[:, j, D + 2 : 2 * D + 2], in0=pg[:, 0:D], scalar1=rcol
                    )

                # --- row path: [A |rowsum rowsum| Bt] = e_row^T @ [Qt|1 1|G]
                oab = pool_out.tile([128, NK, 2 * D + 2], BF16, tag="oab")
                for k in range(NK):
                    pab = pp_gr.tile([128, 2 * D + 2], F32, tag="pgr")
                    for j in range(MJ):
                        nc.tensor.matmul(
                            pab,
                            e_row[:, j, k * 128 : (k + 1) * 128],
                            qtg[:, j, :],
                            start=(j == 0),
                            stop=(j == MJ - 1),
                        )
                    rrow = pool_sm.tile([128, 1], F32, tag=f"rrow{k}")
                    nc.vector.reciprocal(out=rrow, in_=pab[:, D : D + 1])
                    nc.vector.tensor_scalar_mul(
                        out=oab[:, k, :], in0=pab, scalar1=rrow
                    )
                    gsz = 2 if b == bpc - 1 else 4
                    if k % gsz == gsz - 1:
                        g0, g1 = k - gsz + 1, k + 1
                        nc.sync.dma_start(
                            out=A8[b].rearrange("(k p) d -> p k d", p=128)[
                                :, g0:g1, :
                            ],
                            in_=oab[:, g0:g1, 0:D],
                        )
                        nc.sync.dma_start(
                            out=B8[b].rearrange("(k p) d -> p k d", p=128)[
                                :, g0:g1, :
                            ],
                            in_=oab[:, g0:g1, D + 2 : 2 * D + 2],
                        )
    nc.finalize()
    return nc


_NC_CACHE = None


def kernel(C, Q, W0, b0, _trace=False):
    global _NC_CACHE
    if _NC_CACHE is None:
        _NC_CACHE = build_kernel()
    nc = _NC_CACHE

    C = np.ascontiguousarray(np.asarray(C, dtype=np.float32))
    Q = np.ascontiguousarray(np.asarray(Q, dtype=np.float32))
    W0 = np.ascontiguousarray(np.asarray(W0, dtype=np.float32))

    in_maps = [
        {
            "C": C[i * BPC : (i + 1) * BPC],
            "Q": Q[i * BPC : (i + 1) * BPC],
            "W0": W0,
        }
        for i in range(NCORES)
    ]
    res = run_bass_kernel_spmd(nc, in_maps, core_ids=list(range(NCORES)))
    A = np.concatenate(
        [res.results[i]["A"] for i in range(NCORES)], axis=0
    ).astype(np.float32)
    Bt = np.concatenate(
        [res.results[i]["Bt"] for i in range(NCORES)], axis=0
    ).astype(np.float32)
    return (A, Bt)
